# revision 50
# baseline (speedup 1.0000x reference)
"""DH-SFNN Trainium2 kernel (8 NeuronCores, data-parallel over batch).

Model: 2 dendritic LIF layers (K=4 branches, reset-by-subtraction) + leaky
readout integrator, T=250 steps, B=256, IN=700, H=256, O=20.

Fast path (the path that runs for spike-free inputs):
  Spike corrections are strictly subtractive, so if the no-spike membrane
  trajectory never crosses VTH there are exactly zero spikes.  The no-spike
  membrane m^ is a convex combination of past D_sum values (D_sum = K-sum
  of the unit-gain beta-filtered branch currents), hence
  max_t m^ <= max(0, max_t D_sum).  The device verifies max_t D_sum < 0.55
  (Bound B; true value here is 0.307); layer 2's no-spike bound under s1=0
  is the batch-independent bias trajectory, checked exactly on host in f64
  (0.173 here).  When both hold, the output is the closed form
  out[o] = br-driven readout integral (bru), batch-independent.

  Device pipeline per core (BL=32 batches, slabs of 4, time PAIRED into
  NM=128 even/odd column pairs so the DVE scan runs at half length):
    e[m] = (beta*W)x[2m] + W*x[2m+1]   -- fp8e4 DoubleRow matmuls (pair dim
           = time parity, 0.5 cyc/col); weights pre-scaled (1-beta)*2^7 in
           fp8, bias via contraction row 700; ones in x bias row.
    u[2m+1] = beta^2 u[2m-1] + e[m]    -- DVE tensor_tensor_scan, f32
           beta'^2 multiplier slab (exact pole; generated on device from a
           [128,8] column table), fp8 out (one-shot 6% store error only --
           the recurrence feedback is the scan's internal f32).
    D^_odd  = sum_k 8*u                -- fp8 DoubleRow selector matmul
    D^_even = sum_k 8*beta_k*u[m-1] + sum_k 8*(1-beta_k)*2^7*W x[2m]
           -- shifted selector + branch-collapsed direct matmul (the branch
           mask makes each input hit exactly one branch per neuron), both
           accumulated into one PSUM group.  Exact reconstruction, no scan.
    chk: any(D^ > 0.55*2^10) via ACT relu(D^-thr)+accum (PSUM-legal);
         the raw per-(partition, slab*hh) count matrix is DMA'd out as the
         flag and summed on host (no on-device reduce in the tail).
  Scale: D^ = 2^10 * D_sum.  Error budget: fp8 weights (<=8%), fp8 u store
  (<=6%), fp8 selectors (<=6%), pole exact (f32) -- all relative, so
  D^ < 0.55 implies true max D_sum < ~0.8 < VTH.  For the actual inputs the
  computed D^ is 0.306*2^10, 1.8x below threshold; fp8 overflow -> inf/nan
  -> flag != 0 -> safe fallback.

Slow path (flag raised or host checks fail): the original full kernel with
sequential 250-step spike-correction loops (exact, bf16/f32).
"""
import sys

sys.path.insert(0, "/opt/trn_rl_repo")

import numpy as np
import ml_dtypes

import concourse.bass as bass
import concourse.mybir as mybir
import concourse.tile as tile
from concourse import bacc, bass_utils, bass_isa

F32 = mybir.dt.float32
BF16 = mybir.dt.bfloat16
FP8 = mybir.dt.float8e4
ALU = mybir.AluOpType
DRM = mybir.MatmulPerfMode.DoubleRow
FP8NP = mybir.dt.np(FP8)

N_CORES = 8
B, T, IN, H, O, K = 256, 250, 700, 256, 20, 4
BL = B // N_CORES            # 32 batch per core
BBLK = 4                     # batches per scan slab
NBB = BL // BBLK             # 8 slabs
TP = 256                     # padded time (250 real + 6 zero)
KK = 3                       # 768 = 3*256 contraction double-chunks
NF = H * K                   # 1024 branch features
NCF = NF // 128              # 8 feature chunks
VTH = 1.0
WS = 2.0 ** 7                # weight scale (fp8 range)
SELS = 2.0 ** 3              # selector scale -> D^ = 2^10 * D
THR = 0.55 * WS * SELS       # device threshold on D^


def _sig(v):
    return 1.0 / (1.0 + np.exp(-np.asarray(v, np.float64)))


# --------------------------------------------------------------- fast build
# Time-paired scheme: NM = 128 pair-columns per batch (m -> t = 2m, 2m+1),
# u computed at odd t by a beta^2 scan over e[m] = (betaW)x[2m] + Wx[2m+1]
# (one DoubleRow matmul); even-t D reconstructed exactly on PE as
# D[2m] = sum_k beta*w*u[2m-1] (shifted selector) + sum_k w(1-beta)c[2m]
# (branch-collapsed direct matmul; the branch mask makes each input hit
# exactly one branch per neuron, so the collapsed weight is a relabeling).
NM = TP // 2                 # 128 pair columns per batch
NSM = BBLK * NM              # 512 scan columns per slab
IC6 = 6                      # plain 128-row contraction chunks for c1


def build_fast(cps_bufs=4, dps_bufs=2, xs_bufs=3, MIDPTS=(1, 5), OWN6=False):
    nc = bacc.Bacc("TRN2", target_bir_lowering=False, debug=False,
                   num_devices=N_CORES)
    dt = nc.dram_tensor
    # x paired even/odd:      [p, ic, parity, b, m]
    xt_d = dt("xt", [128, IC6, 2, BL, NM], FP8, kind="ExternalInput").ap()
    # c1 weights (beta-paired), quarter-outermost: [p, q, ic, parity, jq]
    w8_d = dt("w8", [128, 4 * IC6 * 2 * 256], FP8, kind="ExternalInput").ap()
    # T2 weights (branch-collapsed): [p, kk, half, hh, h]
    wt2_d = dt("wt2", [128, KK * 2 * 2 * 128], FP8, kind="ExternalInput").ap()
    # beta'^2 per-chunk columns; slab generated on device
    qcol_d = dt("qcol", [128, NCF], F32, kind="ExternalInput").ap()
    # selectors: [p, which(2), hh, call, slot, h]
    sel_d = dt("sel8", [128, 2 * 2 * 2 * 2 * 128], FP8,
               kind="ExternalInput").ap()
    bru_d = dt("brub", [O, BL], F32, kind="ExternalInput").ap()
    out_d = dt("out", [O, BL], F32, kind="ExternalOutput").ap()
    flag_d = dt("flag", [128, 2 * NBB], F32, kind="ExternalOutput").ap()

    with tile.TileContext(nc) as tc:
        with tc.tile_pool(name="const", bufs=1) as cpool, \
             tc.tile_pool(name="xs", bufs=xs_bufs) as xpool, \
             tc.tile_pool(name="us", bufs=3) as upool, \
             tc.tile_pool(name="jk", bufs=3) as jpool, \
             tc.tile_pool(name="small", bufs=1) as mpool:

            def xs_dma(bb, split=False):
                t_ = xpool.tile([128, IC6, 2, NSM], FP8,
                                name=f"xs{bb}", tag="xs")
                tv = t_.rearrange("p i s (b m) -> p i s b m", b=BBLK)
                hb = BBLK // 2 if split else BBLK
                nc.sync.dma_start(
                    out=tv[:, :, :, 0:hb, :],
                    in_=xt_d[:, :, :, bb * BBLK:bb * BBLK + hb, :])
                if split:
                    nc.sync.dma_start(
                        out=tv[:, :, :, hb:, :],
                        in_=xt_d[:, :, :, bb * BBLK + hb:(bb + 1) * BBLK, :])
                return t_

            w8sb = cpool.tile([128, 4, IC6, 2, 256], FP8, name="w8sb")
            w8dv = w8_d.rearrange("p (q a b c) -> p q a b c", q=4, a=IC6, b=2)
            # first quarter (cf 0-1) first, so c1 starts early
            nc.sync.dma_start(out=w8sb[:, 0], in_=w8dv[:, 0])
            xs_pre = {0: xs_dma(0)}
            nc.sync.dma_start(out=w8sb[:, 1], in_=w8dv[:, 1])
            qcol = mpool.tile([128, NCF], F32, name="qcol")
            nc.sync.dma_start(out=qcol, in_=qcol_d)
            bslsb = cpool.tile([128, NCF, BBLK, NM], F32, name="bslsb")
            bslv = bslsb.rearrange("p a b m -> p (a b m)")

            def bsl_gen(cf):
                # broadcast beta^2 column across the slab (ACT), zero the
                # boundary columns (gpsimd) -- keeps DVE free for scans
                nc.scalar.activation(
                    out=bslsb[:, cf, :, :],
                    in_=qcol[:, cf:cf + 1].unsqueeze(2)
                        .broadcast_to((128, BBLK, NM)),
                    func=mybir.ActivationFunctionType.Copy)
                nc.gpsimd.memset(bslsb[:, cf, :, 0:1], 0.0)
                nc.gpsimd.memset(bslsb[:, cf, :, (T - 1) // 2 + 1:], 0.0)

            bsl_gen(0)
            bsl_gen(1)
            nc.sync.dma_start(out=w8sb[:, 2], in_=w8dv[:, 2])
            nc.sync.dma_start(out=w8sb[:, 3], in_=w8dv[:, 3])
            bsl_gen(2)
            bsl_gen(3)
            xs_pre[1] = xs_dma(1)
            for cf in range(4, NCF):
                bsl_gen(cf)
            wt2sb = cpool.tile([128, KK, 2, 2, 128], FP8, name="wt2sb")
            nc.sync.dma_start(out=wt2sb.rearrange("p a b c d -> p (a b c d)"),
                              in_=wt2_d)
            selsb = cpool.tile([128, 2, 2, 2, 2, 128], FP8, name="selsb")
            nc.sync.dma_start(
                out=selsb.rearrange("p a b c d e -> p (a b c d e)"),
                in_=sel_d)
            brusb = mpool.tile([O, BL], F32, name="brusb")
            nc.sync.dma_start(out=brusb, in_=bru_d)
            nc.sync.dma_start(out=out_d, in_=brusb)

            cnt = mpool.tile([128, 2 * NBB], F32, name="cnt")
            thrneg = mpool.tile([128, 1], F32, name="thrneg")
            nc.vector.memset(thrneg, -float(THR))

            with tc.tile_pool(name="cps", bufs=cps_bufs, space="PSUM") as cppool, \
                 tc.tile_pool(name="dps", bufs=dps_bufs, space="PSUM") as dpool:
                def emit_chunks(bb, mid=None):
                    xs = xs_pre.pop(bb) if bb in xs_pre else xs_dma(bb)
                    # u tiles, flat: col 0 is a zero pad so the even-path
                    # selector reads u[m-1] with a plain -1 view; the slot-1
                    # shifted read lands on slot-0's last (junk-zero) column
                    us = [upool.tile([128, 2 * NSM + 2], FP8,
                                     name=f"u{bb}_{pp}", tag=f"u{pp}")
                          for pp in range(NCF // 2)]
                    for pp in range(NCF // 2):
                        nc.gpsimd.memset(us[pp][:, 0:1], 0.0)
                    hold[bb] = (us, xs)
                    def do_chunk(cf):
                        cps = cppool.tile([128, NSM], F32,
                                          name=f"c{bb}_{cf}", tag="cps")
                        for nn in range(2):
                            for ic in range(IC6):
                                nc.tensor.matmul(
                                    cps[:, nn * 256:(nn + 1) * 256],
                                    lhsT=w8sb[:, cf // 2, ic, :,
                                              (cf % 2) * 128:
                                              (cf % 2) * 128 + 128],
                                    rhs=xs[:, ic, :, nn * 256:(nn + 1) * 256],
                                    start=(ic == 0), stop=(ic == IC6 - 1),
                                    perf_mode=DRM)
                        nc.vector.tensor_tensor_scan(
                            out=us[cf // 2][:, 1 + (cf % 2) * NSM:
                                            1 + (cf % 2 + 1) * NSM],
                            data0=bslv[:, cf * NSM:(cf + 1) * NSM],
                            data1=cps,
                            initial=0.0, op0=ALU.mult, op1=ALU.add)

                    for cf in range(NCF):
                        do_chunk(cf)
                        if mid is not None:
                            mid(cf)
                    return us, xs

                def emit_sel(bb, us, xs, only_hh=None):
                    for hh in ((0, 1) if only_hh is None else (only_hh,)):
                        dboth = dpool.tile([128, 2 * NSM], F32,
                                           name=f"d{bb}_{hh}", tag="dps")
                        dod = dboth[:, 0:NSM]
                        dev = dboth[:, NSM:2 * NSM]
                        for nn in range(2):
                            n0 = nn * 256
                            for call in range(2):
                                nc.tensor.matmul(
                                    dod[:, n0:n0 + 256],
                                    lhsT=selsb[:, 0, hh, call, :, :],
                                    rhs=us[hh * 2 + call][:, 1:2 * NSM + 1]
                                        .rearrange("p (s m) -> p s m", s=2)
                                        [:, :, n0:n0 + 256],
                                    start=(call == 0), stop=(call == 1),
                                    perf_mode=DRM)
                            for kk in range(KK):
                                nc.tensor.matmul(
                                    dev[:, n0:n0 + 256],
                                    lhsT=wt2sb[:, kk, :, hh, :],
                                    rhs=xs[:, 2 * kk:2 * kk + 2, 0,
                                           n0:n0 + 256],
                                    start=(kk == 0), stop=False,
                                    perf_mode=DRM)
                            for call in range(2):
                                nc.tensor.matmul(
                                    dev[:, n0:n0 + 256],
                                    lhsT=selsb[:, 1, hh, call, :, :],
                                    rhs=us[hh * 2 + call][:, 0:2 * NSM]
                                        .rearrange("p (s m) -> p s m", s=2)
                                        [:, :, n0:n0 + 256],
                                    start=False, stop=(call == 1),
                                    perf_mode=DRM)
                        col = bb * 2 + hh
                        junk = jpool.tile([128, 2 * NSM], BF16,
                                          name=f"j{bb}_{hh}", tag="junk")
                        nc.scalar.activation(
                            out=junk, in_=dboth,
                            func=mybir.ActivationFunctionType.Relu,
                            bias=thrneg[:, 0:1],
                            accum_out=cnt[:, col:col + 1])

                prev = None
                hold = {}
                for bb in range(NBB):
                    last = bb == NBB - 1
                    if prev is not None:
                        pb, pu, px = bb - 1, prev[0], prev[1]

                        def mid(cf, pb=pb, pu=pu, px=px, last=last, bb=bb):
                            if cf == MIDPTS[0]:
                                emit_sel(pb, pu, px, only_hh=0)
                            elif cf == MIDPTS[1]:
                                emit_sel(pb, pu, px, only_hh=1)
                            elif last and OWN6 and cf == 6:
                                emit_sel(bb, *hold[bb], only_hh=0)
                    else:
                        mid = None
                    cur = emit_chunks(bb, mid)
                    prev = cur
                if OWN6:
                    emit_sel(NBB - 1, *prev, only_hh=1)
                else:
                    emit_sel(NBB - 1, *prev)

            nc.sync.dma_start(out=flag_d, in_=cnt)

    nc.compile()
    return nc


# ------------------------------------------------------------ fast host prep
def prep_inputs(x, W1, b1, tau_n1, tau_m1, W2, b2, tau_n2, tau_m2,
                Wr, br, tau_mr, warmup):
    """Host-side inputs for the fast (spike-bound check) kernel."""
    w = int(np.asarray(warmup))
    beta = _sig(tau_n1).reshape(NF)                        # [NF] f64
    alpha1 = _sig(tau_m1)                                  # [H]
    alphar = _sig(tau_mr)                                  # [O]
    W1T = np.zeros((KK * 256, NF), np.float64)             # [768, NF]
    W1T[:IN] = np.asarray(W1, np.float64).T
    W1T[IN] = np.asarray(b1, np.float64)

    # c1 pair weights: parity0 = beta*(1-beta)*W*2^7, parity1 = (1-beta)*W*2^7
    gw = (1.0 - beta) * WS
    w8 = np.stack([W1T * (beta * gw), W1T * gw], axis=1)   # [768, 2, NF]
    w8 = np.ascontiguousarray(
        w8.reshape(IC6, 128, 2, 4, 256).transpose(1, 3, 0, 2, 4)
    ).astype(FP8NP).reshape(128, 4 * IC6 * 2 * 256)

    # T2 weights: branch-collapsed sum_k W[h4k,i]*(1-beta_k)*2^10
    g2w = (1.0 - beta) * WS * SELS
    V = (W1T * g2w) @ np.kron(np.eye(H), np.ones((K, 1)))  # [768, H]
    wt2 = np.ascontiguousarray(
        V.reshape(KK, 2, 128, 2, 128).transpose(2, 0, 1, 3, 4)
    ).astype(FP8NP).reshape(128, KK * 2 * 2 * 128)

    # beta'^2 per-chunk columns (f32); the slab is generated on device
    q = (beta.astype(np.float32).astype(np.float64) ** 2).astype(np.float32)
    qcol = np.ascontiguousarray(q.reshape(NCF, 128).T)

    # selectors: which=0 odd (w=8), which=1 even-T1 (beta_j*8).  No
    # (1-alpha) factor: soundness needs max_t sum_k u_k <= VTH (m^ is a
    # convex combination of D_sum values).
    selw = np.full(H, SELS)
    sel8 = np.zeros((128, 2, 2, 2, 2, 128), np.float64)
    p = np.arange(128)
    for cf in range(NCF):
        hh, call, slot = cf // 4, (cf % 4) // 2, cf % 2
        h = cf * 32 + p // 4
        j = cf * 128 + p
        sel8[p, 0, hh, call, slot, h - hh * 128] = selw[h]
        sel8[p, 1, hh, call, slot, h - hh * 128] = beta[j] * selw[h]
    sel8 = sel8.astype(FP8NP).reshape(128, 2 * 2 * 2 * 2 * 128)

    # closed-form readout for the zero-spike case: out = br * U / (T-w)
    tt = np.arange(T, dtype=np.float64)[:, None]
    ar = alphar[None, :]
    u = ar ** np.maximum(0, w - tt) - ar ** (T - tt)              # [T, O]
    bru = np.asarray(br, np.float64) * u.sum(0) / (T - w)         # [O]
    brub = np.tile(bru[:, None], (1, BL)).astype(np.float32)

    # x in fp8 (0/1 exact), bias row of ones, zero padding
    xx = np.zeros((KK * 256, B, TP), np.float32)
    xx[:IN, :, :T] = np.asarray(x, np.float32).transpose(2, 0, 1)
    xx[IN, :, :T] = 1.0
    xev = xx[:, :, 0::2]                                   # [768, B, NM]
    xod = xx[:, :, 1::2]
    xt = np.stack([xev, xod], axis=1)                      # [768, 2, B, NM]
    xt = np.ascontiguousarray(
        xt.reshape(IC6, 128, 2, B, NM).transpose(1, 0, 2, 3, 4)
    ).astype(FP8NP)                                        # [128,6,2,B,NM]

    shared = dict(w8=w8, wt2=wt2, qcol=qcol, sel8=sel8, brub=brub)
    in_maps = []
    for c in range(N_CORES):
        m = dict(shared)
        m["xt"] = np.ascontiguousarray(xt[:, :, :, c * BL:(c + 1) * BL, :])
        in_maps.append(m)
    return in_maps


def host_checks(x, W2, b2, tau_n2, tau_m2, **_):
    """Exact f64 checks that don't need the device: finite inputs, and the
    layer-2 no-spike bound under s1=0 (bias-driven trajectory only)."""
    if not np.isfinite(np.asarray(x)).all():
        return False
    beta2 = _sig(tau_n2).reshape(NF)
    alpha2 = _sig(tau_m2)
    b2g = np.asarray(b2, np.float64)
    d = np.zeros(NF)
    m = np.zeros(H)
    mx = -np.inf
    for _t in range(T):
        d = beta2 * d + (1.0 - beta2) * b2g
        m = m * alpha2 + (1.0 - alpha2) * d.reshape(H, K).sum(-1)
        mx = max(mx, m.max())
    return mx < 0.9 * VTH


_NC_CACHE = {}


def get_nc(slow=False, **kw):
    key = "slow" if slow else "fast" + repr(sorted(kw.items()))
    if key not in _NC_CACHE:
        _NC_CACHE[key] = build_slow() if slow else build_fast(**kw)
    return _NC_CACHE[key]


def kernel(**inputs):
    out = np.empty((B, O), np.float32)
    if host_checks(**{k: inputs[k] for k in
                      ("x", "W2", "b2", "tau_n2", "tau_m2")}):
        in_maps = prep_inputs(**inputs)
        res = bass_utils.run_bass_kernel_spmd(
            get_nc(), in_maps, core_ids=list(range(N_CORES)))
        if all(float(np.asarray(r["flag"]).sum()) == 0.0
               for r in res.results):
            for c in range(N_CORES):
                out[c * BL:(c + 1) * BL] = res.results[c]["out"].T
            return out
    # spikes possible: run the exact kernel with the correction loops
    in_maps = prep_inputs_slow(**inputs)
    res = bass_utils.run_bass_kernel_spmd(
        get_nc(slow=True), in_maps, core_ids=list(range(N_CORES)))
    for c in range(N_CORES):
        out[c * BL:(c + 1) * BL] = res.results[c]["out"].T
    return out


# ======================================================================
# Slow path: original exact kernel (spike-correction loops), unchanged.
# ======================================================================
BBLK_S = 4
NSL_S = BBLK_S * T           # 1000 slab columns
IC = 6                       # 768 = 6*128 contraction chunks (row 700 = bias)
NN_SPLITS = [(0, 512), (512, 488)]


def build_slow():
    nc = bacc.Bacc("TRN2", target_bir_lowering=False, debug=False,
                   num_devices=N_CORES)
    dt = nc.dram_tensor
    xt_d = dt("xt", [IC * 128, BL, T], BF16, kind="ExternalInput").ap()
    w1_d = dt("w1p", [IC * 128, NF], BF16, kind="ExternalInput").ap()
    w2_d = dt("w2p", [H, NF], BF16, kind="ExternalInput").ap()
    wr_d = dt("wrt", [128, 2 * O], BF16, kind="ExternalInput").ap()
    m2b_d = dt("mh2b", [128, 2 * T], BF16, kind="ExternalInput").ap()
    bsl1_d = dt("bsl1", [NCF, 128, NSL_S], BF16, kind="ExternalInput").ap()
    bsl2_d = dt("bsl2", [NCF, 128, NSL_S], BF16, kind="ExternalInput").ap()
    asl_d = dt("asl", [128, 4 * NSL_S], BF16, kind="ExternalInput").ap()
    acol_d = dt("acol", [128, 4], F32, kind="ExternalInput").ap()
    sel_d = dt("selm", [128, 32], BF16, kind="ExternalInput").ap()
    ur_d = dt("ur", [O, T], F32, kind="ExternalInput").ap()
    bru_d = dt("bru", [O, 1], F32, kind="ExternalInput").ap()
    out_d = dt("out", [O, BL], F32, kind="ExternalOutput").ap()
    flag_d = dt("flag", [128, 2 * NBB], F32, kind="ExternalOutput").ap()

    with tile.TileContext(nc) as tc:
        with tc.tile_pool(name="const", bufs=1) as cpool, \
             tc.tile_pool(name="state", bufs=1) as spool, \
             tc.tile_pool(name="bsl", bufs=1) as bpool, \
             tc.tile_pool(name="xs", bufs=2) as xpool, \
             tc.tile_pool(name="ds", bufs=2) as dpool, \
             tc.tile_pool(name="small", bufs=1) as mpool:

            w1sb = [cpool.tile([128, NF], BF16, name=f"w1sb{i}", tag=f"w1_{i}")
                    for i in range(IC)]
            for i in range(IC):
                nc.sync.dma_start(out=w1sb[i], in_=w1_d[i * 128:(i + 1) * 128, :])
            w2sb = [cpool.tile([128, NF], BF16, name=f"w2sb{i}", tag=f"w2_{i}")
                    for i in range(2)]
            for i in range(2):
                nc.sync.dma_start(out=w2sb[i], in_=w2_d[i * 128:(i + 1) * 128, :])
            wrsb = cpool.tile([128, 2 * O], BF16, name="wrsb")
            nc.sync.dma_start(out=wrsb, in_=wr_d)
            m2bsb = cpool.tile([128, 2 * T], BF16, name="m2bsb")
            nc.sync.dma_start(out=m2bsb, in_=m2b_d)
            aslsb = cpool.tile([128, 4 * NSL_S], BF16, name="aslsb")
            nc.sync.dma_start(out=aslsb, in_=asl_d)
            acolsb = cpool.tile([128, 4], F32, name="acolsb")
            nc.sync.dma_start(out=acolsb, in_=acol_d)
            selsb = cpool.tile([128, 32], BF16, name="selsb")
            nc.sync.dma_start(out=selsb, in_=sel_d)
            ursb = cpool.tile([O, T], F32, name="ursb")
            nc.sync.dma_start(out=ursb, in_=ur_d)
            brusb = cpool.tile([O, 1], F32, name="brusb")
            nc.sync.dma_start(out=brusb, in_=bru_d)

            mhat = spool.tile([128, 2 * NBB * NSL_S], BF16, name="mhat")
            sfull = spool.tile([128, 2 * NBB * NSL_S], BF16, name="sfull")
            q = mpool.tile([128, 64], BF16, name="q")
            cnt = mpool.tile([128, 4], F32, name="cnt")
            csum = mpool.tile([128, 2], F32, name="csum")
            par = mpool.tile([128, 2], F32, name="par")
            acc = mpool.tile([O, BL], F32, name="acc")
            accb = mpool.tile([O, BL], F32, name="accb")
            zjunk = mpool.tile([O, T], F32, name="zjunk")

            mh_v = mhat.rearrange("p (hh b t) -> p hh b t", hh=2, b=BL, t=T)
            sf_v = sfull.rearrange("p (hh b t) -> p hh b t", hh=2, b=BL, t=T)
            q_v = q.rearrange("p (hh b) -> p hh b", hh=2)

            with tc.tile_pool(name="psA", bufs=2, space="PSUM") as pspool:

                def layer(L, bsl_d, rhs_mm):
                    bslsb = bpool.tile([128, NCF * NSL_S], BF16,
                                       name=f"bslsb{L}", tag="bsl")
                    for cf in range(NCF):
                        nc.sync.dma_start(out=bslsb[:, cf * NSL_S:(cf + 1) * NSL_S],
                                          in_=bsl_d[cf])
                    aoff = (L - 1) * 2 * NSL_S
                    for bb in range(NBB):
                        ds = dpool.tile([128, NCF * NSL_S], BF16,
                                        name=f"ds{L}_{bb}", tag="ds")
                        for cf in range(NCF):
                            ps = pspool.tile([128, NSL_S], F32,
                                             name=f"c{L}_{bb}_{cf}", tag="mm")
                            for nn in range(2):
                                rhs_mm(ps, bb, cf, nn)
                            nc.vector.tensor_tensor_scan(
                                out=ds[:, cf * NSL_S:(cf + 1) * NSL_S],
                                data0=bslsb[:, cf * NSL_S:(cf + 1) * NSL_S],
                                data1=ps,
                                initial=0.0, op0=ALU.mult, op1=ALU.add)
                        for hh in range(2):
                            Dps = pspool.tile([128, 1024], F32,
                                              name=f"D{L}_{bb}_{hh}", tag="D")
                            for c4 in range(4):
                                o4 = (hh * 4 + c4) * NSL_S
                                for n0, nw in NN_SPLITS:
                                    nc.tensor.matmul(
                                        Dps[c4 * 32:(c4 + 1) * 32,
                                            n0:n0 + nw],
                                        lhsT=selsb,
                                        rhs=ds[:, o4 + n0:o4 + n0 + nw],
                                        start=True, stop=True,
                                        tile_position=(0, c4 * 32))
                            nc.vector.tensor_tensor_scan(
                                out=mhat[:, hh * 8000 + bb * NSL_S:
                                         hh * 8000 + (bb + 1) * NSL_S],
                                data0=aslsb[:, aoff + hh * NSL_S:
                                            aoff + (hh + 1) * NSL_S],
                                data1=Dps[:, 0:NSL_S], initial=0.0,
                                op0=ALU.mult, op1=ALU.add)

                def spike_phase(L):
                    nc.gpsimd.memset(sfull, 0.0)
                    junk = dpool.tile([128, NCF * NSL_S], BF16,
                                      name=f"junk{L}", tag="ds")
                    for hh in range(2):
                        nc.vector.tensor_scalar(
                            out=junk[:, 0:8000],
                            in0=mhat[:, hh * 8000:(hh + 1) * 8000],
                            scalar1=float(VTH), scalar2=None, op0=ALU.is_gt,
                            op1=ALU.add,
                            accum_out=cnt[:, (L - 1) * 2 + hh:(L - 1) * 2 + hh + 1])
                    nc.vector.tensor_add(
                        out=csum[:, L - 1:L],
                        in0=cnt[:, (L - 1) * 2:(L - 1) * 2 + 1],
                        in1=cnt[:, (L - 1) * 2 + 1:(L - 1) * 2 + 2])
                    nc.gpsimd.partition_all_reduce(
                        par[:, L - 1:L], csum[:, L - 1:L], channels=128,
                        reduce_op=bass_isa.ReduceOp.add)
                    nc.vector.memset(q, 0.0)
                    for t in range(T):
                        nc.vector.scalar_tensor_tensor(
                            out=sf_v[:, :, :, t], in0=mh_v[:, :, :, t],
                            scalar=float(VTH), op0=ALU.subtract,
                            in1=q_v, op1=ALU.is_gt)
                        for hh in range(2):
                            nc.vector.scalar_tensor_tensor(
                                out=q[:, hh * 32:(hh + 1) * 32],
                                in0=q[:, hh * 32:(hh + 1) * 32],
                                scalar=acolsb[:, (L - 1) * 2 + hh:
                                              (L - 1) * 2 + hh + 1],
                                op0=ALU.mult,
                                in1=sf_v[:, hh, :, t], op1=ALU.add)

                xs = {}

                def mm1(ps, bb, cf, nn):
                    n0, nw = NN_SPLITS[nn]
                    if cf == 0 and nn == 0:
                        for i in range(IC):
                            t_ = xpool.tile([128, NSL_S], BF16,
                                            name=f"xs{bb}_{i}", tag=f"xs{i}")
                            nc.sync.dma_start(
                                out=t_.rearrange("p (b t) -> p b t", b=BBLK_S),
                                in_=xt_d[i * 128:(i + 1) * 128,
                                         bb * BBLK_S:(bb + 1) * BBLK_S, :])
                            xs[i] = t_
                    for i in range(IC):
                        nc.tensor.matmul(
                            ps[:, n0:n0 + nw],
                            lhsT=w1sb[i][:, cf * 128:(cf + 1) * 128],
                            rhs=xs[i][:, n0:n0 + nw],
                            start=(i == 0), stop=(i == IC - 1))

                layer(1, bsl1_d, mm1)
                spike_phase(1)

                def mm2(ps, bb, cf, nn):
                    n0, nw = NN_SPLITS[nn]
                    for hh in range(2):
                        nc.tensor.matmul(
                            ps[:, n0:n0 + nw],
                            lhsT=w2sb[hh][:, cf * 128:(cf + 1) * 128],
                            rhs=sfull[:, hh * 8000 + bb * NSL_S + n0:
                                      hh * 8000 + bb * NSL_S + n0 + nw],
                            start=(hh == 0), stop=(hh == 1))

                layer(2, bsl2_d, mm2)
                nc.vector.tensor_add(
                    out=mh_v, in0=mh_v,
                    in1=m2bsb.rearrange("p (hh t) -> p hh t", hh=2)
                        .unsqueeze(2).broadcast_to((128, 2, BL, T)))
                spike_phase(2)

            with tc.tile_pool(name="psB", bufs=2, space="PSUM") as zpool:
                for bb in range(NBB):
                    for nn in range(2):
                        zps = zpool.tile([O, 500], F32, name=f"z{bb}_{nn}",
                                         tag="z")
                        for hh in range(2):
                            nc.tensor.matmul(
                                zps,
                                lhsT=wrsb[:, hh * O:(hh + 1) * O],
                                rhs=sfull[:, hh * 8000 + bb * NSL_S + nn * 500:
                                          hh * 8000 + bb * NSL_S + (nn + 1) * 500],
                                start=(hh == 0), stop=(hh == 1))
                        for b2 in range(2):
                            b = bb * BBLK_S + nn * 2 + b2
                            nc.vector.scalar_tensor_tensor(
                                out=zjunk, in0=zps[:, b2 * T:(b2 + 1) * T],
                                scalar=1.0, op0=ALU.mult,
                                in1=ursb, op1=ALU.mult,
                                accum_out=acc[:, b:b + 1])
                nc.vector.tensor_scalar(
                    out=accb, in0=acc, scalar1=brusb[:, 0:1], scalar2=None,
                    op0=ALU.add)
                nc.sync.dma_start(out=out_d, in_=accb)
                nc.sync.dma_start(out=flag_d, in_=par[0:1, 0:2])

    nc.compile()
    return nc


def prep_inputs_slow(x, W1, b1, tau_n1, tau_m1, W2, b2, tau_n2, tau_m2,
                     Wr, br, tau_mr, warmup):
    w = int(np.asarray(warmup))
    beta1 = _sig(tau_n1).reshape(NF)
    alpha1 = _sig(tau_m1)
    beta2 = _sig(tau_n2).reshape(NF)
    alpha2 = _sig(tau_m2)
    alphar = _sig(tau_mr)

    g1 = (1.0 - beta1) * np.repeat(1.0 - alpha1, K)
    g2 = (1.0 - beta2) * np.repeat(1.0 - alpha2, K)

    w1p = np.zeros((IC * 128, NF), np.float64)
    w1p[:IN] = np.asarray(W1, np.float64).T * g1
    w1p[IN] = np.asarray(b1, np.float64) * g1
    w1p = w1p.astype(ml_dtypes.bfloat16)

    w2p = (np.asarray(W2, np.float64).T * g2).astype(ml_dtypes.bfloat16)
    b2g = np.asarray(b2, np.float64) * g2
    dtraj = np.zeros(NF)
    mh2b = np.zeros((H, T))
    mtraj = np.zeros(H)
    for t_ in range(T):
        dtraj = _sig(tau_n2).reshape(NF) * dtraj + b2g
        mtraj = _sig(tau_m2) * mtraj + dtraj.reshape(H, K).sum(-1)
        mh2b[:, t_] = mtraj
    mh2b_dev = np.zeros((128, 2 * T), np.float64)
    mh2b_dev[:, :T] = mh2b[:128]
    mh2b_dev[:, T:] = mh2b[128:]
    mh2b_dev = mh2b_dev.astype(ml_dtypes.bfloat16)

    wrt = np.zeros((128, 2 * O), np.float64)
    wrt[:, :O] = np.asarray(Wr, np.float64).T[:128]
    wrt[:, O:] = np.asarray(Wr, np.float64).T[128:]
    wrt = wrt.astype(ml_dtypes.bfloat16)

    def bslab(beta):
        s = np.tile(beta.reshape(NCF, 128, 1).astype(ml_dtypes.bfloat16),
                    (1, 1, NSL_S))
        s.reshape(NCF, 128, BBLK_S, T)[:, :, :, 0] = 0.0
        return s

    bsl1 = bslab(beta1)
    bsl2 = bslab(beta2)

    def aslab(alpha):
        a2 = alpha.reshape(2, 128).astype(ml_dtypes.bfloat16)
        s = np.tile(a2[:, :, None], (1, 1, NSL_S))
        s.reshape(2, 128, BBLK_S, T)[:, :, :, 0] = 0.0
        return s

    asl = np.concatenate([aslab(alpha1), aslab(alpha2)], axis=0)
    asl = asl.transpose(1, 0, 2).reshape(128, 4 * NSL_S).copy()

    acol = np.stack([alpha1[:128], alpha1[128:], alpha2[:128], alpha2[128:]],
                    axis=1).astype(np.float32)

    selm = np.zeros((128, 32), ml_dtypes.bfloat16)
    selm[np.arange(128), np.arange(128) // 4] = 1.0

    tt = np.arange(T, dtype=np.float64)[:, None]
    ar = alphar[None, :]
    u = ar ** np.maximum(0, w - tt) - ar ** (T - tt)
    ur = (u.T / (T - w)).astype(np.float32)
    bru = (np.asarray(br, np.float64) * u.sum(0) / (T - w)) \
        .astype(np.float32)[:, None]

    xt_full = np.zeros((IC * 128, B, T), ml_dtypes.bfloat16)
    xt_full[:IN] = np.asarray(x).transpose(2, 0, 1)
    xt_full[IN] = 1.0

    shared = dict(w1p=w1p, w2p=w2p, mh2b=mh2b_dev, wrt=wrt,
                  bsl1=bsl1, bsl2=bsl2, asl=asl, acol=acol, selm=selm,
                  ur=ur, bru=bru)
    in_maps = []
    for c in range(N_CORES):
        m = dict(shared)
        m["xt"] = np.ascontiguousarray(xt_full[:, c * BL:(c + 1) * BL, :])
        in_maps.append(m)
    return in_maps
